# revision 1
# baseline (speedup 1.0000x reference)
"""Causal MHA with RoPE on 8 TRN2 NeuronCores.

Sharding: data-parallel over batch (2) x tensor-parallel over heads (4 groups
of 4 heads) = 8 cores. Core c handles batch c//4, head group c%4.
Each core computes its 4 heads' attention and a partial output projection
(Wo sharded row-wise); host sums the 4 partials per batch.

Per-core device algorithm (all matmuls in float32r = TF32, fp32 accumulate):
  - QK^T projection: qkT[dk, s] = (Wqk rows).T-contracted with xT (host-transposed x)
  - RoPE applied on [dk(partition), s] layout via cos/sin tables and a
    stream_shuffle partition pair-swap
  - scores^T[k, q] = K^T.T-free @ Q^T per head (K=64 contraction, two heads
    packed in row groups 0-1 / 2-3 of the PE array)
  - probsT = exp(scores/8) straight from PSUM (no max subtraction; scores are
    N(0,1)-scaled so exp never overflows), causal tri-mask on diagonal tiles
  - attnT_unnorm[dk, q] (+ row of sums via a ones column in [V|1]) = [V|1].T @ probsT
  - softmax normalization: sums row -> K=1 broadcast matmul -> reciprocal ->
    one elementwise multiply
  - partial out = attnT.T-contracted with WoT chunks, accumulated over the
    2 head pairs, DMA'd to DRAM
"""
import sys
import os

for _p in ("/opt/trn_rl_repo", "/root/.axon_site/_ro/trn_rl_repo"):
    if os.path.isdir(_p) and _p not in sys.path:
        sys.path.insert(0, _p)

import numpy as np

import concourse.mybir as mybir
import concourse.tile as tile
from concourse import bacc
from concourse.bass_utils import run_bass_kernel_spmd

F32 = mybir.dt.float32
F32R = mybir.dt.float32r
AF = mybir.ActivationFunctionType
MULT = mybir.AluOpType.mult
ADD = mybir.AluOpType.add

B, S, D = 2, 2048, 1024
H, DK = 16, 64
THETA = 10000.0
NCORES = 8
GROUPS = 4          # head groups (tensor parallel)
GH = H // GROUPS    # heads per group = 4
GF = GH * DK        # features per group = 256
SWAP_MASK = [i ^ 1 for i in range(32)]
KVER = 14  # bump on any kernel change: busts the HLO-shape-keyed NEFF cache

_CACHED = {}


def _build_nc(iters=1):
    _iters = iters
    nc = bacc.Bacc("TRN2", target_bir_lowering=False, debug=False, num_devices=NCORES)
    xT = nc.dram_tensor("xT", [D, S], F32R, kind="ExternalInput").ap()
    wqkT = nc.dram_tensor("wqkT", [D, 2 * GF], F32R, kind="ExternalInput").ap()
    wvT = nc.dram_tensor("wvT", [D, GF], F32R, kind="ExternalInput").ap()
    woT = nc.dram_tensor("woT", [GF, D], F32R, kind="ExternalInput").ap()
    cosf = nc.dram_tensor("cosf", [128, S], F32, kind="ExternalInput").ap()
    sins = nc.dram_tensor("sins", [128, S], F32, kind="ExternalInput").ap()
    tri = nc.dram_tensor("tri", [128, 128], F32, kind="ExternalInput").ap()
    ho = nc.dram_tensor("ho", [1, 256], F32R, kind="ExternalInput").ap()
    onesc = nc.dram_tensor("onesc", [128, 1], F32R, kind="ExternalInput").ap()
    # unused input whose shape encodes the kernel version: the neuron compile
    # cache keys on HLO structure only, so two kernels with identical I/O
    # shapes would otherwise collide.
    nc.dram_tensor("cachebust", [iters, KVER], F32, kind="ExternalInput")
    out = nc.dram_tensor("out", [S, D], F32, kind="ExternalOutput").ap()

    SB = S // 512  # 4 q-tiles of 512
    KB = S // 128  # 16 k-blocks of 128

    with tile.TileContext(nc) as tc:
        with tc.tile_pool(name="const", bufs=1) as cpool, \
             tc.tile_pool(name="xt", bufs=8) as xpool, \
             tc.tile_pool(name="big", bufs=1) as bpool, \
             tc.tile_pool(name="work", bufs=2) as wpool, \
             tc.tile_pool(name="probs", bufs=4) as ppool, \
             tc.tile_pool(name="psum", bufs=1, space="PSUM") as psum:

            # ---- loads, ordered by first use on the single HWDGE queue ----
            wqk_sb = cpool.tile([128, 8, 2 * GF], F32R, tag="wqk")
            wv_sb = cpool.tile([128, 8, GF], F32R, tag="wv")
            wo_sb = cpool.tile([128, 2, D], F32R, tag="wo")
            cos_sb = cpool.tile([128, S], F32, tag="cos")
            sin_sb = cpool.tile([128, S], F32, tag="sin")
            tri_sb = cpool.tile([128, 128], F32, tag="tri")
            ho_sb = cpool.tile([1, 256], F32R, tag="ho")
            onesc_sb = cpool.tile([128, 1], F32R, tag="onesc")
            xt_tiles = []
            for dc in range(8):
                t = xpool.tile([128, S], F32R, tag="xt")
                xt_tiles.append(t)

            for dc in range(8):  # interleaved so MM(dc) unblocks asap
                nc.sync.dma_start(wqk_sb[:, dc, :], wqkT[dc * 128:(dc + 1) * 128, :])
                nc.sync.dma_start(xt_tiles[dc][:, 0:512],
                                  xT[dc * 128:(dc + 1) * 128, 0:512])
            nc.sync.dma_start(cos_sb[:, 0:1024], cosf[:, 0:1024])
            nc.sync.dma_start(sin_sb[:, 0:1024], sins[:, 0:1024])
            nc.sync.dma_start(onesc_sb[:], onesc)
            nc.sync.dma_start(ho_sb[:], ho)
            nc.sync.dma_start(tri_sb[:], tri)
            for dc in range(8):  # t=1 quarter
                nc.sync.dma_start(xt_tiles[dc][:, 512:1024],
                                  xT[dc * 128:(dc + 1) * 128, 512:1024])
            nc.sync.dma_start(wv_sb[:], wvT.rearrange("(dc p) n -> p dc n", p=128))
            nc.sync.dma_start(cos_sb[:, 1024:S], cosf[:, 1024:S])
            nc.sync.dma_start(sin_sb[:, 1024:S], sins[:, 1024:S])
            nc.sync.dma_start(wo_sb[:], woT.rearrange("(fc p) n -> p fc n", p=128))
            for dc in range(8):  # t=2,3 half
                nc.sync.dma_start(xt_tiles[dc][:, 1024:S],
                                  xT[dc * 128:(dc + 1) * 128, 1024:S])

            warm = cpool.tile([1, 1], F32, tag="warm")
            nc.scalar.activation(warm[:], ho_sb[0:1, 0:1], AF.Exp, scale=1.0)

            # ---- phase 1: QK projection + RoPE ----
            # qkT slabs: 0,1 = Q head-pairs; 2,3 = K head-pairs
            for _it in range(iters):
              qkT = bpool.tile([128, 4, S], F32R, tag="qkT", name=f"qkT{_it}")
              vt = bpool.tile([128, KB, GH, DK + 1], F32R, tag="vt", name=f"vt{_it}")
              nc.vector.tensor_copy(
                  vt[:, :, :, DK:DK + 1],
                  onesc_sb[:, None, None, :].to_broadcast([128, KB, GH, 1]))
              for t in range(SB):
                  for c in range(4):
                      ps = psum.tile([128, 512], F32, tag="sc", bufs=2)
                      for dc in range(8):
                          nc.tensor.matmul(
                              ps[:], wqk_sb[:, dc, c * 128:(c + 1) * 128],
                              xt_tiles[dc][:, t * 512:(t + 1) * 512],
                              start=(dc == 0), stop=(dc == 7))
                      tsl = slice(t * 512, (t + 1) * 512)
                      # rope: qkT = ps*cos + swap(ps*sins)
                      nc.vector.tensor_tensor(qkT[:, c, tsl], ps[:], cos_sb[:, tsl], MULT)
                      tmp = wpool.tile([128, 512], F32, tag="ropetmp")
                      nc.vector.tensor_tensor(tmp[:], ps[:], sin_sb[:, tsl], MULT)
                      tmp2 = wpool.tile([128, 512], F32, tag="ropetmp2")
                      nc.vector.stream_shuffle(tmp2[:], tmp[:], SWAP_MASK)
                      nc.gpsimd.tensor_tensor(qkT[:, c, tsl], qkT[:, c, tsl], tmp2[:], ADD)
                  # V projection for this t's 4 s-blocks (interleaved with QK)
                  for sb_i in range(4 * t, 4 * t + 4):
                      psv = psum.tile([128, GF], F32, tag="sc", bufs=2)
                      for dc in range(8):
                          nc.tensor.matmul(
                              psv[:], xt_tiles[dc][:, sb_i * 128:(sb_i + 1) * 128],
                              wv_sb[:, dc, :], start=(dc == 0), stop=(dc == 7))
                      nc.scalar.copy(
                          vt[:, sb_i, :, 0:DK],
                          psv[:].rearrange("p (h d) -> p h d", h=GH))

              # ---- phase 3: attention per head pair ----
              attnT = [cpool.tile([128, S], F32R, tag=("cos" if p == 0 else "sin"),
                                  name=f"attnT{p}_{_it}") for p in range(2)]
              for qt in range(SB):
                  for pair in range(2):
                      qs, ks = pair, 2 + pair
                      pva = psum.tile([128, 512], F32, tag="pv", bufs=2)
                      pvb = psum.tile([128, 512], F32, tag="pv", bufs=2)
                      nkb = 4 * qt + 4
                      for kb in range(nkb):
                          lam = max(kb - 4 * qt, 0) * 128
                          qsl = slice(qt * 512 + lam, (qt + 1) * 512)
                          ksl = slice(kb * 128, (kb + 1) * 128)
                          ss = psum.tile([128, 2, 512], F32, tag="sc2", bufs=2)
                          nc.tensor.matmul(ss[:, 0, lam:512], qkT[0:64, ks, ksl],
                                           qkT[0:64, qs, qsl], start=True, stop=True)
                          nc.tensor.matmul(ss[:, 1, lam:512], qkT[64:128, ks, ksl],
                                           qkT[64:128, qs, qsl], start=True, stop=True)
                          pab = ppool.tile([128, 2, 512], F32R, tag="probs")
                          nc.scalar.activation(pab[:, :, lam:512], ss[:, :, lam:512], AF.Exp, scale=0.125)
                          if kb >= 4 * qt:  # diagonal block: causal tri mask
                              dsl = slice(lam, lam + 128)
                              nc.gpsimd.tensor_tensor(
                                  pab[:, :, dsl], pab[:, :, dsl],
                                  tri_sb[:, None, :].to_broadcast([128, 2, 128]), MULT)
                          nc.tensor.matmul(pva[0:65, lam:512], vt[:, kb, 2 * pair, :],
                                           pab[:, 0, lam:512], start=(kb == 0), stop=(kb == nkb - 1))
                          nc.tensor.matmul(pvb[0:65, lam:512], vt[:, kb, 2 * pair + 1, :],
                                           pab[:, 1, lam:512], start=(kb == 0), stop=(kb == nkb - 1))
                      # normalization: sums row 64 -> bcast -> recip -> multiply
                      qtsl = slice(qt * 512, (qt + 1) * 512)
                      sra = wpool.tile([1, 512], F32R, tag="srow")
                      srb = wpool.tile([1, 512], F32R, tag="srow2")
                      nc.vector.tensor_copy(sra[:], pva[64:65, :])
                      nc.vector.tensor_copy(srb[:], pvb[64:65, :])
                      psr = psum.tile([128, 512], F32, tag="sc", bufs=2)
                      nc.tensor.matmul(psr[:], ho_sb[:, 0:128], sra[:], start=True, stop=False)
                      nc.tensor.matmul(psr[:], ho_sb[:, 128:256], srb[:], start=False, stop=True)
                      rbc = wpool.tile([128, 512], F32, tag="rbc_sb")
                      nc.vector.reciprocal(rbc[:], psr[:])
                      nc.vector.tensor_copy(attnT[pair][0:64, qtsl], pva[0:64, :])
                      nc.vector.tensor_copy(attnT[pair][64:128, qtsl], pvb[0:64, :])
                      nc.vector.tensor_tensor(attnT[pair][:, qtsl], attnT[pair][:, qtsl],
                                              rbc[:], MULT)

                  # ---- output projection for this qt's q-blocks (interleaved) ----
                  for qb in range(4 * qt, 4 * qt + 4):
                      qsl = slice(qb * 128, (qb + 1) * 128)
                      osb = xpool.tile([128, D], F32, tag="xt", name=f"osb{qb}_{_it}")
                      for nh in range(2):
                          nsl = slice(nh * 512, (nh + 1) * 512)
                          pso = psum.tile([128, 512], F32, tag="sc", bufs=2)
                          nc.tensor.matmul(pso[:], attnT[0][:, qsl], wo_sb[:, 0, nsl],
                                           start=True, stop=False)
                          nc.tensor.matmul(pso[:], attnT[1][:, qsl], wo_sb[:, 1, nsl],
                                           start=False, stop=True)
                          nc.vector.tensor_copy(osb[:, nsl], pso[:])
                      nc.sync.dma_start(out[qsl, :], osb[:])

    nc.compile()
    return nc


def _host_tables(token_positions):
    pos = np.asarray(token_positions, dtype=np.float32)  # [S]
    half = DK // 2
    freq = THETA ** (-np.arange(0, DK, 2, dtype=np.float32) / DK)  # [32]
    # per-partition tables on [dk(128 = 2 heads of 64), s]
    f64 = np.repeat(freq, 2)          # [64] freq per feature index
    ang64 = pos[None, :] * f64[:, None]  # [64, S]
    cos64 = np.cos(ang64)
    sin64 = np.sin(ang64)
    sign = np.where(np.arange(DK) % 2 == 0, 1.0, -1.0).astype(np.float32)  # +s even, -s odd
    sins64 = sin64 * sign[:, None]
    cosf = np.concatenate([cos64, cos64], axis=0).astype(np.float32)   # [128, S]
    sins = np.concatenate([sins64, sins64], axis=0).astype(np.float32)  # [128, S]
    return cosf, sins


def kernel(x, Wq, Wk, Wv, Wo, token_positions):
    x = np.asarray(x, dtype=np.float32)
    Wq = np.asarray(Wq, dtype=np.float32)
    Wk = np.asarray(Wk, dtype=np.float32)
    Wv = np.asarray(Wv, dtype=np.float32)
    Wo = np.asarray(Wo, dtype=np.float32)

    if "nc" not in _CACHED:
        _CACHED["nc"] = _build_nc(iters=int(os.environ.get("BENCH_ITERS", "1")))
    nc = _CACHED["nc"]

    cosf, sins = _host_tables(token_positions)
    tri = np.triu(np.ones((128, 128), dtype=np.float32))  # tri[k, j] = 1 if j >= k
    ho = np.concatenate([
        np.concatenate([np.ones(64), np.zeros(64)]),
        np.concatenate([np.zeros(64), np.ones(64)]),
    ]).astype(np.float32)[None, :]  # [1, 256]
    onesc = np.ones((128, 1), dtype=np.float32)

    xT = [np.ascontiguousarray(x[b].T) for b in range(B)]  # [D, S] each
    in_maps = []
    for c in range(NCORES):
        b, g = c // GROUPS, c % GROUPS
        R = slice(g * GF, (g + 1) * GF)
        wqkT = np.ascontiguousarray(
            np.concatenate([Wq[R].T, Wk[R].T], axis=1))  # [D, 512]
        wvT = np.ascontiguousarray(Wv[R].T)              # [D, 256]
        woT = np.ascontiguousarray(Wo[:, R].T)           # [256, D]
        in_maps.append({
            "xT": xT[b], "wqkT": wqkT, "wvT": wvT, "woT": woT,
            "cosf": cosf, "sins": sins, "tri": tri, "ho": ho, "onesc": onesc,
            "cachebust": np.zeros((int(os.environ.get("BENCH_ITERS", "1")), KVER), dtype=np.float32),
        })

    try:
        res = run_bass_kernel_spmd(nc, in_maps, core_ids=list(range(NCORES)))
    except Exception:
        # transient NRT_EXEC_UNIT_UNRECOVERABLE flakes recover on retry
        import time as _time
        _time.sleep(2.0)
        res = run_bass_kernel_spmd(nc, in_maps, core_ids=list(range(NCORES)))
    _CACHED["last_results"] = res
    outs = [r["out"] for r in res.results]  # each [S, D] partial
    full = np.empty((B, S, D), dtype=np.float32)
    for b in range(B):
        full[b] = sum(outs[b * GROUPS + g] for g in range(GROUPS))
    return full



# revision 44
# speedup vs baseline: 1.2221x; 1.2221x over previous
"""Causal MHA with RoPE on 8 TRN2 NeuronCores.

Sharding: data-parallel over batch (2) x tensor-parallel over heads (4 groups
of 4 heads) = 8 cores. Core c handles batch c//4, head group c%4.
Each core computes its 4 heads' attention and a partial output projection
(Wo sharded row-wise); host sums the 4 partials per batch.

v3 design (pipelined bf16):
  - All SBUF operands bf16 (PSUM stays fp32): matmul rate is 1 cycle/row
    either way, but bf16 removes the fp32r free<256 4x penalty on diagonal
    blocks, halves DMA bytes, and unlocks DVE 2x perf modes.
  - Host pre-arranges every tensor in its exact SBUF layout.
  - Single fused loop over tiles t=0..3 (512 tokens each):
      QK-proj(t)+RoPE -> V-proj(t) -> outproj(t-1) -> attention(qt=t)
    so TensorE never waits on a softmax chain: outproj is deferred one
    tile, and the softmax normalization has no PE instructions at all
    (reciprocal on DVE + partition_broadcast on Pool).
  - attention: scores^T[k,q] per head pair in one PSUM tile [128,2,512],
    exp from PSUM (scale=1/8) -> pab bf16, tri-mask on diagonal (DVE 2x),
    PV with ones-column in vt for softmax sums.
  - engine budget: PE ~116us, Act ~74us (exp + vt copies in proj phase),
    DVE ~74us (rope muls/shuffle, masks, recip, attnT scale, psum->osb),
    Pool ~32us (rope adds, reciprocal broadcast).
"""
import sys
import os

for _p in ("/opt/trn_rl_repo", "/root/.axon_site/_ro/trn_rl_repo"):
    if os.path.isdir(_p) and _p not in sys.path:
        sys.path.insert(0, _p)

import numpy as np
import ml_dtypes

import concourse.mybir as mybir
import concourse.tile as tile
from concourse import bacc
from concourse.bass_utils import run_bass_kernel_spmd

F32 = mybir.dt.float32
F32R = mybir.dt.float32r
BF16 = mybir.dt.bfloat16
NPBF = ml_dtypes.bfloat16
AF = mybir.ActivationFunctionType
MULT = mybir.AluOpType.mult
ADD = mybir.AluOpType.add

B, S, D = 2, 2048, 1024
H, DK = 16, 64
THETA = 10000.0
NCORES = 8
GROUPS = 4          # head groups (tensor parallel)
GH = H // GROUPS    # heads per group = 4
GF = GH * DK        # features per group = 256
SWAP_MASK = [i ^ 1 for i in range(32)]
KVER = 24  # bump on any kernel change: busts the HLO-shape-keyed NEFF cache

_CACHED = {}

SB = S // 512  # 4 tiles of 512 tokens
KB = S // 128  # 16 k-blocks of 128


def _build_nc(iters=1):
    nc = bacc.Bacc("TRN2", target_bir_lowering=False, debug=False, num_devices=NCORES)
    # host pre-arranged, all bf16, exact SBUF layouts
    xt_d = nc.dram_tensor("xt", [128, 8, S], BF16, kind="ExternalInput").ap()
    wqk_d = nc.dram_tensor("wqk", [128, 8, 2 * GF], BF16, kind="ExternalInput").ap()
    wv_d = nc.dram_tensor("wv", [128, 8, GF], BF16, kind="ExternalInput").ap()
    wo_d = nc.dram_tensor("wo", [128, 2, D], BF16, kind="ExternalInput").ap()
    cos_d = nc.dram_tensor("cosf", [128, S], BF16, kind="ExternalInput").ap()
    sin_d = nc.dram_tensor("sins", [128, S], BF16, kind="ExternalInput").ap()
    tri_d = nc.dram_tensor("tri", [128, 128], BF16, kind="ExternalInput").ap()
    onesc_d = nc.dram_tensor("onesc", [128, 1], BF16, kind="ExternalInput").ap()
    # unused input whose shape encodes the kernel version: the neuron compile
    # cache keys on HLO structure only, so two kernels with identical I/O
    # shapes would otherwise collide.
    nc.dram_tensor("cachebust", [iters, KVER], F32, kind="ExternalInput")
    out = nc.dram_tensor("out", [S, D], BF16, kind="ExternalOutput").ap()

    with tile.TileContext(nc) as tc:
        with tc.tile_pool(name="const", bufs=1) as cpool, \
             tc.tile_pool(name="work", bufs=3) as wpool, \
             tc.tile_pool(name="probs", bufs=6) as ppool, \
             tc.tile_pool(name="outb", bufs=6) as opool, \
             tc.tile_pool(name="psum", bufs=1, space="PSUM") as psum:

            xt = cpool.tile([128, 8, S], BF16, tag="xt")
            wqk = cpool.tile([128, 8, 2 * GF], BF16, tag="wqk")
            wv = cpool.tile([128, 8, GF], BF16, tag="wv")
            wo = cpool.tile([128, 2, D], BF16, tag="wo")
            cos_sb = cpool.tile([128, S], BF16, tag="cos")
            sin_sb = cpool.tile([128, S], BF16, tag="sin")
            tri_sb = cpool.tile([128, 128], BF16, tag="tri")
            onesc_sb = cpool.tile([128, 1], BF16, tag="onesc")

            # ---- loads: few big DMAs (each pays ~625ns serialized HWDGE);
            # the first tile's wqk/xt stream in dc-pair chunks so the
            # (dc-outer) first projection starts after the first pair ----
            for h in range(4):
                hs = slice(2 * h, 2 * h + 2)
                nc.sync.dma_start(wqk[:, hs, :], wqk_d[:, hs, :])
                nc.sync.dma_start(xt[:, hs, 0:512], xt_d[:, hs, 0:512])
            nc.sync.dma_start(onesc_sb[:], onesc_d)
            nc.sync.dma_start(cos_sb[:, 0:512], cos_d[:, 0:512])
            nc.sync.dma_start(sin_sb[:, 0:512], sin_d[:, 0:512])
            nc.sync.dma_start(wv[:], wv_d)
            nc.sync.dma_start(tri_sb[:], tri_d)
            nc.sync.dma_start(xt[:, :, 512:1024], xt_d[:, :, 512:1024])
            nc.sync.dma_start(cos_sb[:, 512:S], cos_d[:, 512:S])
            nc.sync.dma_start(sin_sb[:, 512:S], sin_d[:, 512:S])
            nc.sync.dma_start(wo[:], wo_d)
            nc.sync.dma_start(xt[:, :, 1024:1536], xt_d[:, :, 1024:1536])
            nc.sync.dma_start(xt[:, :, 1536:S], xt_d[:, :, 1536:S])

            warm = cpool.tile([1, 1], F32, tag="warm")
            nc.scalar.activation(warm[:], onesc_sb[0:1, 0:1], AF.Exp, scale=1.0)

            for _it in range(iters):
                qkT = cpool.tile([128, 4, S], BF16, tag="qkT", name=f"qkT{_it}")
                vt = cpool.tile([128, KB, GH, DK + 1], BF16, tag="vt", name=f"vt{_it}")
                attnT = [cpool.tile([128, S], BF16, tag=f"attnT{p}",
                                    name=f"attnT{p}_{_it}") for p in range(2)]
                nc.vector.tensor_copy(
                    vt[:, :, :, DK:DK + 1],
                    onesc_sb[:, None, None, :].to_broadcast([128, KB, GH, 1]))

                pend = []   # deferred pair-1 normalization args
                for t in range(SB):
                    tsl = slice(t * 512, (t + 1) * 512)
                    # ---- QK projection + RoPE, tile t ----
                    for c in range(4):
                        ps = psum.tile([128, 512], F32, tag="mm", bufs=2)
                        for dc in range(8):
                            nc.tensor.matmul(
                                ps[:], wqk[:, dc, c * 128:(c + 1) * 128],
                                xt[:, dc, tsl],
                                start=(dc == 0), stop=(dc == 7))
                        # rope now, while c+1 projects
                        _rope(nc, wpool, qkT, ps[:], cos_sb, sin_sb, c, tsl)
                    # ---- V projection, tile t (4 s-blocks) ----
                    for sb_i in range(4 * t, 4 * t + 4):
                        psv = psum.tile([128, 512], F32, tag="mm", bufs=2)
                        for dc in range(8):
                            nc.tensor.matmul(
                                psv[:, 0:GF], xt[:, dc, sb_i * 128:(sb_i + 1) * 128],
                                wv[:, dc, :], start=(dc == 0), stop=(dc == 7))
                        nc.scalar.copy(
                            vt[:, sb_i, :, 0:DK],
                            psv[:, 0:GF].rearrange("p (h d) -> p h d", h=GH))

                    # deferred pair-1 normalization of the previous tile:
                    # its DVE ops land after rope(t) in the DVE queue, so
                    # they never block next-tile projection work.
                    if pend:
                        _norm(nc, wpool, attnT, *pend.pop())

                    # ---- attention for qt = t; the deferred outproj of tile
                    # t-1 is split 2+2 around the pair loop so PE has work
                    # while each pair's softmax-normalization chain drains ----
                    for pair in range(2):
                        if t > 0:
                            _outproj(nc, psum, opool, attnT, wo, out, t - 1,
                                     qbs=range(4 * (t - 1) + 2 * pair,
                                               4 * (t - 1) + 2 * pair + 2))
                        qs, ks = pair, 2 + pair
                        pva = psum.tile([128, 512], F32, tag="pv", bufs=2)
                        pvb = psum.tile([128, 512], F32, tag="pv", bufs=2)
                        nkb = 4 * t + 4
                        # software-pipelined kb loop: scores/exp of kb+1 are
                        # emitted before PV of kb, so PE never sits waiting
                        # for the Act engine's exp of the current block.
                        pabs = {}

                        def _produce(kb, t=t, pair=pair, qs=qs, ks=ks):
                            lam = max(kb - 4 * t, 0) * 128
                            qsl = slice(t * 512 + lam, (t + 1) * 512)
                            ksl = slice(kb * 128, (kb + 1) * 128)
                            ss = psum.tile([128, 2, 512], F32, tag="ss", bufs=2,
                                           name=f"ss_{_it}_{t}_{pair}_{kb}")
                            nc.tensor.matmul(ss[:, 0, lam:512], qkT[0:64, ks, ksl],
                                             qkT[0:64, qs, qsl], start=True, stop=True)
                            nc.tensor.matmul(ss[:, 1, lam:512], qkT[64:128, ks, ksl],
                                             qkT[64:128, qs, qsl], start=True, stop=True)
                            pab = ppool.tile([128, 2, 512], BF16, tag="probs",
                                             name=f"pab_{_it}_{t}_{pair}_{kb}")
                            nc.scalar.activation(pab[:, :, lam:512], ss[:, :, lam:512],
                                                 AF.Exp, scale=0.125)
                            if kb >= 4 * t:  # diagonal block: causal tri mask
                                dsl = slice(lam, lam + 128)
                                nc.vector.tensor_tensor(
                                    pab[:, :, dsl], pab[:, :, dsl],
                                    tri_sb[:, None, :].to_broadcast([128, 2, 128]), MULT)
                            pabs[kb] = (pab, lam)

                        def _pv(kb, nkb=nkb, t=t, pair=pair, pva=pva, pvb=pvb):
                            pab, lam = pabs.pop(kb)
                            nc.tensor.matmul(pva[0:65, lam:512], vt[:, kb, 2 * pair, :],
                                             pab[:, 0, lam:512], start=(kb == 0),
                                             stop=(kb == nkb - 1))
                            nc.tensor.matmul(pvb[0:65, lam:512], vt[:, kb, 2 * pair + 1, :],
                                             pab[:, 1, lam:512], start=(kb == 0),
                                             stop=(kb == nkb - 1))

                        LA = 3  # scores lookahead depth (ss bufs cap issue at 2 ahead)
                        for kb in range(min(LA, nkb)):
                            _produce(kb)
                        for kb in range(LA, nkb):
                            _produce(kb)
                            _pv(kb - LA)
                        for kb in range(max(nkb - LA, 0), nkb):
                            _pv(kb)
                        # normalization: no PE instructions — recip the sums
                        # rows (65th row of pva/pvb) and partition-broadcast.
                        # pair 1's norm is deferred into the next tile so its
                        # DVE ops can't stall next-tile PE work via psum WARs.
                        if pair == 0 or t == SB - 1:
                            _norm(nc, wpool, attnT, pva, pvb, pair, t)
                        else:
                            pend.append((pva, pvb, pair, t))

                # keep the PE p-state ramped while the final softmax-norm
                # chain drains: a dozen 1-row matmuls nobody reads.
                jp = psum.tile([128, 2, 512], F32, tag="ss", bufs=2,
                               name=f"jp_{_it}")
                for i in range(20):
                    nc.tensor.matmul(jp[0:1, i % 2, :], onesc_sb[:, 0:1],
                                     qkT[:, 0, 0:512], start=True, stop=True,
                                     skip_group_check=True)

                # final tile's output projection (per-nh DMAs shorten the tail)
                _outproj(nc, psum, opool, attnT, wo, out, SB - 1, split_dma=True)

    nc.compile()
    return nc


def _rope(nc, wpool, qkT, ps, cos_sb, sin_sb, c, tsl):
    # qkT[:, c, tsl] = ps*cos + partition-pair-swap(ps*sins)
    nc.vector.tensor_tensor(qkT[:, c, tsl], ps, cos_sb[:, tsl], MULT)
    tmp = wpool.tile([128, 512], BF16, tag="ropetmp")
    nc.vector.tensor_tensor(tmp[:], ps, sin_sb[:, tsl], MULT)
    tmp2 = wpool.tile([128, 512], BF16, tag="ropetmp2")
    nc.vector.stream_shuffle(tmp2[:], tmp[:], SWAP_MASK)
    nc.gpsimd.tensor_tensor(qkT[:, c, tsl], qkT[:, c, tsl], tmp2[:], ADD)


def _norm(nc, wpool, attnT, pva, pvb, pair, t):
    # partition_broadcast only works with a partition-0-based destination on
    # hardware, so each head's reciprocal row gets its own [64, 512] tile.
    qtsl = slice(t * 512, (t + 1) * 512)
    rbca = wpool.tile([64, 512], F32, tag="rbca")
    rbcb = wpool.tile([64, 512], F32, tag="rbcb")
    ra = wpool.tile([1, 512], F32, tag="ra")
    rb = wpool.tile([1, 512], F32, tag="rb")
    nc.vector.reciprocal(ra[:], pva[64:65, :])
    nc.vector.reciprocal(rb[:], pvb[64:65, :])
    nc.gpsimd.partition_broadcast(rbca[:], ra[:])
    nc.gpsimd.partition_broadcast(rbcb[:], rb[:])
    nc.vector.tensor_tensor(attnT[pair][0:64, qtsl], pva[0:64, :],
                            rbca[:], MULT)
    nc.vector.tensor_tensor(attnT[pair][64:128, qtsl], pvb[0:64, :],
                            rbcb[:], MULT)


def _outproj(nc, psum, opool, attnT, wo, out, t, qbs=None, split_dma=False):
    for qb in (qbs if qbs is not None else range(4 * t, 4 * t + 4)):
        qsl = slice(qb * 128, (qb + 1) * 128)
        osb = opool.tile([128, D], BF16, tag="osb")
        for nh in range(2):
            nsl = slice(nh * 512, (nh + 1) * 512)
            pso = psum.tile([128, 512], F32, tag="mm", bufs=2)
            nc.tensor.matmul(pso[:], attnT[0][:, qsl], wo[:, 0, nsl],
                             start=True, stop=False)
            nc.tensor.matmul(pso[:], attnT[1][:, qsl], wo[:, 1, nsl],
                             start=False, stop=True)
            nc.vector.tensor_copy(osb[:, nsl], pso[:])
            if split_dma:
                nc.sync.dma_start(out[qsl, nsl], osb[:, nsl])
        if not split_dma:
            nc.sync.dma_start(out[qsl, :], osb[:])


def _host_tables(token_positions):
    pos = np.asarray(token_positions, dtype=np.float32)  # [S]
    freq = THETA ** (-np.arange(0, DK, 2, dtype=np.float32) / DK)  # [32]
    f64 = np.repeat(freq, 2)          # [64] freq per feature index
    ang64 = pos[None, :] * f64[:, None]  # [64, S]
    cos64 = np.cos(ang64)
    sin64 = np.sin(ang64)
    sign = np.where(np.arange(DK) % 2 == 0, 1.0, -1.0).astype(np.float32)
    sins64 = sin64 * sign[:, None]
    cosf = np.concatenate([cos64, cos64], axis=0)   # [128, S]
    sins = np.concatenate([sins64, sins64], axis=0)  # [128, S]
    return cosf.astype(NPBF), sins.astype(NPBF)


def kernel(x, Wq, Wk, Wv, Wo, token_positions):
    x = np.asarray(x, dtype=np.float32)
    Wq = np.asarray(Wq, dtype=np.float32)
    Wk = np.asarray(Wk, dtype=np.float32)
    Wv = np.asarray(Wv, dtype=np.float32)
    Wo = np.asarray(Wo, dtype=np.float32)

    if "nc" not in _CACHED:
        _CACHED["nc"] = _build_nc(iters=int(os.environ.get("BENCH_ITERS", "1")))
    nc = _CACHED["nc"]

    cosf, sins = _host_tables(token_positions)
    tri = np.triu(np.ones((128, 128), dtype=np.float32)).astype(NPBF)
    onesc = np.ones((128, 1), dtype=NPBF)

    # xt[p, dc, s] = x[b][s, dc*128+p]
    xts = []
    for b in range(B):
        xt = np.ascontiguousarray(
            x[b].T.reshape(8, 128, S).transpose(1, 0, 2)).astype(NPBF)
        xts.append(xt)

    in_maps = []
    for c in range(NCORES):
        b, g = c // GROUPS, c % GROUPS
        R = slice(g * GF, (g + 1) * GF)
        # wqk[p, dc, j]: j in 0:256 -> Wq rows, 256:512 -> Wk rows; contract
        # dim index = dc*128+p
        wqkT = np.concatenate([Wq[R].T, Wk[R].T], axis=1)  # [D, 512]
        wqk = np.ascontiguousarray(
            wqkT.reshape(8, 128, 2 * GF).transpose(1, 0, 2)).astype(NPBF)
        wvT = Wv[R].T                                      # [D, 256]
        wv = np.ascontiguousarray(
            wvT.reshape(8, 128, GF).transpose(1, 0, 2)).astype(NPBF)
        woT = Wo[:, R].T                                   # [256, D]
        wo = np.ascontiguousarray(
            woT.reshape(2, 128, D).transpose(1, 0, 2)).astype(NPBF)
        in_maps.append({
            "xt": xts[b], "wqk": wqk, "wv": wv, "wo": wo,
            "cosf": cosf, "sins": sins, "tri": tri, "onesc": onesc,
            "cachebust": np.zeros((int(os.environ.get("BENCH_ITERS", "1")), KVER),
                                  dtype=np.float32),
        })

    try:
        res = run_bass_kernel_spmd(nc, in_maps, core_ids=list(range(NCORES)))
    except Exception:
        # transient NRT_EXEC_UNIT_UNRECOVERABLE flakes recover on retry
        import time as _time
        _time.sleep(2.0)
        res = run_bass_kernel_spmd(nc, in_maps, core_ids=list(range(NCORES)))
    _CACHED["last_results"] = res
    # each partial is [S, D] bf16; sum in fp32 on host
    outs = [np.asarray(r["out"]).astype(np.float32) for r in res.results]
    full = np.empty((B, S, D), dtype=np.float32)
    for b in range(B):
        full[b] = sum(outs[b * GROUPS + g] for g in range(GROUPS))
    return full
